# revision 30
# baseline (speedup 1.0000x reference)
"""Iter-1 + G1 (pt 4-slot 176-stride + incremental G reduce)."""

import os
import sys

import numpy as np

for _p in ("/opt/trn_rl_repo", "/root/.axon_site/_ro/trn_rl_repo"):
    if os.path.isdir(_p) and _p not in sys.path:
        sys.path.insert(0, _p)

import ml_dtypes

import concourse.bacc as bacc
import concourse.bass as bass  # noqa: F401
import concourse.mybir as mybir
import concourse.tile as tile
from concourse.bass_utils import run_bass_kernel_spmd

F32 = mybir.dt.float32
BF16 = mybir.dt.bfloat16
NPBF16 = ml_dtypes.bfloat16
N, F_IN, HID, C, K = 8192, 512, 64, 16, 10
BASE_ALPHA = 0.5
JA, JB, JL, JR = 1.0, 1.0, -1.0, 1.0
NCORES = 8
SH = N // NCORES
NB = SH // 128
NSB = NB // 2
RCH = N // 128
MYR = SH // 128

_CACHE = {}

def _jacobi_coef_rows(temp):
    """Host-precomputed per-channel coefficient rows, [30*C] packed."""
    a, b, l, r = JA, JB, JL, JR
    alphas = (BASE_ALPHA * np.tanh(np.asarray(temp, np.float64)))  # [C, K+1]
    rows = [alphas[:, 0]]
    coef1 = (a - b) / 2 - (a + b + 2) / 2 * (l + r) / (r - l)
    coef2 = (a + b + 2) / (r - l)
    rows.append(coef1 * alphas[:, 1])   # c1_0
    rows.append(coef2 * alphas[:, 1])   # c1_1
    for L in range(2, K + 1):
        coef_l = 2 * L * (L + a + b) * (2 * L - 2 + a + b)
        c_lm1_1 = (2 * L + a + b - 1) * (2 * L + a + b) * (2 * L + a + b - 2)
        c_lm1_2 = (2 * L + a + b - 1) * (a ** 2 - b ** 2)
        c_lm2 = 2 * (L - 1 + a) * (L - 1 + b) * (2 * L + a + b)
        tmp1 = alphas[:, L - 1] * (c_lm1_1 / coef_l)
        tmp2 = alphas[:, L - 1] * (c_lm1_2 / coef_l)
        tmp3 = alphas[:, L - 1] * alphas[:, L - 2] * (c_lm2 / coef_l)
        rows.append(tmp1 * (2 / (r - l)))                    # t1
        rows.append(tmp1 * ((r + l) / (r - l)) + tmp2)       # t2
        rows.append(tmp3)                                    # t3
    packed = np.concatenate(rows).astype(np.float32).reshape(1, 30 * C)
    return np.ascontiguousarray(np.repeat(packed, 128, axis=0))


def _bc(ap, shape, axis=1):
    """Broadcast an AP to a 3D [128, NB, C]-style shape with stride-0 dims."""
    while ap.ndim < len(shape):
        ap = ap.unsqueeze(axis)
    return ap.broadcast_to(shape)




def _build():
    nc = bacc.Bacc("TRN2", target_bir_lowering=False, debug=False)

    u_sh = nc.dram_tensor("u_shard", [N, SH], BF16, kind="ExternalInput")
    x_sh = nc.dram_tensor("x_shard", [F_IN, N], BF16, kind="ExternalInput")
    e_sh = nc.dram_tensor("e_shard", [MYR, 128], F32, kind="ExternalInput")
    w1r = nc.dram_tensor("w1r", [128, 4 * HID], BF16, kind="ExternalInput")
    w2d = nc.dram_tensor("w2d", [HID, C], BF16, kind="ExternalInput")
    b1c = nc.dram_tensor("b1c", [HID, 1], F32, kind="ExternalInput")
    b2c = nc.dram_tensor("b2c", [C, 1], F32, kind="ExternalInput")
    jcd = nc.dram_tensor("jcd", [128, 30 * C], F32, kind="ExternalInput")
    id128d = nc.dram_tensor("id128d", [128, 128], BF16, kind="ExternalInput")
    id16x4d = nc.dram_tensor("id16x4d", [128, C], F32, kind="ExternalInput")
    idb16d = nc.dram_tensor("idb16d", [128, C], BF16, kind="ExternalInput")
    out_sh = nc.dram_tensor("out_shard", [SH, C], F32, kind="ExternalOutput")

    rg = [list(range(NCORES))]

    with nc.allow_low_precision(reason="bf16 single-pass matmul path"), \
         tile.TileContext(nc) as tc:
        with (
            tc.tile_pool(name="dram", bufs=1, space="DRAM") as dram,
            tc.tile_pool(name="consts", bufs=1) as cp,
            tc.tile_pool(name="persist", bufs=1) as pp,
            tc.tile_pool(name="usb", bufs=12) as up,
            tc.tile_pool(name="utsb", bufs=2) as utp,
            tc.tile_pool(name="small", bufs=4) as sp,
        ):
            rs_in_a = dram.tile([N, C], mybir.dt.float16)
            rs_in_b = dram.tile([N, C], mybir.dt.float16)
            rs_out_a = dram.tile([SH, C], mybir.dt.float16)
            rs_out_b = dram.tile([SH, C], mybir.dt.float16)

            id128 = cp.tile_from(id128d[:])
            id16x4 = cp.tile_from(id16x4d[:])
            idb16 = cp.tile_from(idb16d[:])
            jc = cp.tile_from(jcd[:])
            w1 = cp.tile_from(w1r[:])
            w2 = cp.tile_from(w2d[:])
            b1 = cp.tile_from(b1c[:])
            b2 = cp.tile_from(b2c[:])
            e_row = cp.tile_from(e_sh[:])

            zid = pp.tile([128, RCH, C + 128], BF16)
            gsb = pp.tile([16, 256], F32)
            gacc = pp.tile([128, NB, C], F32)
            e_col = pp.tile([128, MYR], F32)
            hacc = pp.tile([128, NB, C], F32)
            xs_a = pp.tile([128, NB, C], F32)
            xs_b = pp.tile([128, NB, C], F32)
            htmp = pp.tile([128, NB, C], F32)
            htmp2 = pp.tile([128, NB, C], F32)
            accsb_a = pp.tile([128, 2048], BF16)
            accsb_b = pp.tile([128, 2048], BF16)
            o8_a = pp.tile([128, 8, C], mybir.dt.float16)
            o8_b = pp.tile([128, 8, C], mybir.dt.float16)
            smin = pp.tile([128, MYR, C], mybir.dt.float16)
            smb = pp.tile([128, MYR, C], mybir.dt.float16)
            smf = pp.tile([128, MYR, C], F32)
            smout = pp.tile([128, MYR, C], F32)
            nc.gpsimd.dma_start(
                out=zid[:, :, C:C + 128],
                in_=id128[:].unsqueeze(1).broadcast_to((128, RCH, 128)))

            # sb0's U fetches first: no z dependency, keeps DMA + PE busy
            u0_t = []
            for g in range(RCH // 8):
                u_t = up.tile([128, 8, 256], BF16, tag="u")
                nc.sync.dma_start(
                    out=u_t[:],
                    in_=u_sh[1024 * g:1024 * (g + 1), 0:256]
                    .rearrange("(j p) c -> p j c", p=128),
                )
                u0_t.append(u_t)

            # replicated MLP: every core computes the FULL z locally and
            # writes it straight into zid -- no AllGather, no bounce DMAs
            with tc.tile_pool(name="ppre", bufs=1, space="PSUM") as ppre, \
                 tc.tile_pool(name="mlp", bufs=2) as mp:
                pet = ppre.tile([128, MYR], F32, tag="ptmp", bufs=3)
                nc.tensor.transpose(pet[:], e_row[:], id16x4[0:MYR, 0:MYR])
                nc.scalar.copy(e_col[:], pet[:])
                for ch in range(8):
                    xTq = mp.tile([128, 4, 1024], BF16, tag="xq")
                    nc.scalar.dma_start(
                        out=xTq[:],
                        in_=x_sh[:, 1024 * ch:1024 * (ch + 1)]
                        .rearrange("(a p) r -> p a r", p=128))
                    ph = ppre.tile([HID, 1024], F32, tag="ph", bufs=1)
                    for half in range(2):
                        for fb in range(4):
                            nc.tensor.matmul(
                                ph[:, 512 * half:512 * (half + 1)],
                                lhsT=w1[:, fb * HID:(fb + 1) * HID],
                                rhs=xTq[:, fb, 512 * half:512 * (half + 1)],
                                start=(fb == 0), stop=(fb == 3),
                            )
                    hq = mp.tile([HID, 1024], BF16, tag="hq")
                    nc.scalar.activation(hq[:], ph[:],
                                         mybir.ActivationFunctionType.Relu,
                                         bias=b1[:, 0:1], scale=1.0)
                    pz = ppre.tile([C, 1024], F32, tag="pz", bufs=1)
                    for half in range(2):
                        nc.tensor.matmul(pz[:, 512 * half:512 * (half + 1)],
                                         lhsT=w2[:],
                                         rhs=hq[:, 512 * half:512 * (half + 1)],
                                         start=True, stop=True)
                    zTq = mp.tile([C, 1024], F32, tag="zq")
                    nc.vector.tensor_scalar_add(zTq[:], pz[:], b2[:, 0:1])
                    for j in range(8):
                        rc = 8 * ch + j
                        pzt = ppre.tile([128, C], F32, tag="ptmp", bufs=3)
                        nc.tensor.transpose(pzt[:], zTq[:, 128 * j:128 * (j + 1)],
                                            id16x4[0:C, :])
                        if rc % 2 == 0:
                            nc.scalar.copy(zid[:, rc, 0:C], pzt[:])
                        else:
                            nc.vector.tensor_copy(zid[:, rc, 0:C], pzt[:])
            ev = _bc(e_col[:], (128, NB, C), axis=2)

            def jrow(i):
                return _bc(jc[:, i * C:(i + 1) * C], (128, NB, C))

            nc.gpsimd.tensor_copy(xs_a[:], jrow(0))
            nc.gpsimd.tensor_mul(htmp[:], xs_a[:], ev)
            nc.gpsimd.tensor_mul(htmp[:], htmp[:], jrow(2))
            nc.gpsimd.tensor_add(xs_b[:], htmp[:], jrow(1))
            nc.gpsimd.tensor_add(hacc[:], xs_a[:], xs_b[:])
            xm2, xm1 = xs_a, xs_b
            for L in range(2, K + 1):
                r0 = 3 + 3 * (L - 2)
                nc.gpsimd.tensor_mul(htmp[:], xm1[:], ev)
                nc.gpsimd.tensor_mul(htmp[:], htmp[:], jrow(r0))
                nc.gpsimd.tensor_mul(htmp2[:], xm1[:], jrow(r0 + 1))
                nc.gpsimd.tensor_sub(htmp[:], htmp[:], htmp2[:])
                nc.gpsimd.tensor_mul(htmp2[:], xm2[:], jrow(r0 + 2))
                nc.gpsimd.tensor_sub(xm2[:], htmp[:], htmp2[:])
                nc.gpsimd.tensor_add(hacc[:], hacc[:], xm2[:])
                xm2, xm1 = xm1, xm2

            def emit_acc_tail(pm, accsb, o8, rs_in):
                for ch in range(4):
                    dstc = accsb[:, 512 * ch:512 * (ch + 1)]
                    if ch % 2 == 0:
                        nc.scalar.copy(dstc, pacc[:, 512 * ch:512 * (ch + 1)])
                    else:
                        nc.vector.tensor_copy(dstc, pacc[:, 512 * ch:512 * (ch + 1)])
                for rc in range(RCH):
                    kq, jj = rc // 16, rc % 16
                    pt2 = pm.tile([128, 4, C + 128], BF16, tag="pt", bufs=2)
                    nc.tensor.transpose(
                        pt2[:, 0, 0:C],
                        accsb[32 * kq:32 * kq + C, jj * 128:(jj + 1) * 128],
                        idb16[32 * kq:32 * kq + C, :],
                        tile_position=(32 * kq, 0),
                    )
                    dst = o8[:, rc % 8, :]
                    if rc % 2 == 0:
                        nc.scalar.copy(dst, pt2[:, 0, 0:C])
                    else:
                        nc.vector.tensor_copy(dst, pt2[:, 0, 0:C])
                    if rc % 8 == 7:
                        r0 = rc - 7
                        nc.gpsimd.dma_start(
                            out=rs_in[r0 * 128:(r0 + 8) * 128, :]
                            .rearrange("(j p) c -> p j c", p=128),
                            in_=o8[:],
                        )

            with (
                tc.tile_pool(name="pmain", bufs=1, space="PSUM") as pm,
            ):
                pacc = pm.tile([128, 2048], F32, tag="pacc")
                nc.vector.memset(pacc[:], 0.0)
                for sb in range(NSB):
                    ut_t = utp.tile([128, RCH, 2, C + 128], BF16, tag="ut")
                    if sb == 0:
                        # pass T: transpose-only (no z dependency)
                        for g in range(RCH // 8):
                            u_t = u0_t[g]
                            for jj in range(8):
                                rc = 8 * g + jj
                                q = rc % 2
                                if q == 0:
                                    ptb = pm.tile([128, 2, 2, 176], BF16,
                                                  tag="pt", bufs=2)
                                for h in range(2):
                                    nc.tensor.transpose(
                                        ptb[:, q, h, 0:128],
                                        u_t[:, jj, 128 * h:128 * (h + 1)],
                                        id128[:],
                                    )
                                if q == 1:
                                    dstb = ut_t[:, rc - 1:rc + 1, :, C:C + 128]
                                    srcb = ptb[:, :, :, 0:128]
                                    if (rc // 2) % 2 == 0:
                                        nc.scalar.copy(dstb, srcb)
                                    else:
                                        nc.vector.tensor_copy(dstb, srcb)
                        # pass G (after z lands): z-stationary accumulation
                        # single 256-wide group: a start-MM's has_written clear
                        # wipes its partition rows across the whole bank, so
                        # both blocks must share one accumulation region
                        pgt = pm.tile([16, 256], F32, tag="pt", bufs=2)
                        for g in range(RCH // 8):
                            u_t = u0_t[g]
                            for jj in range(8):
                                rc = 8 * g + jj
                                nc.tensor.matmul(
                                    pgt[:],
                                    lhsT=zid[:, rc, 0:C],
                                    rhs=u_t[:, jj, :],
                                    start=(rc == 0), stop=(rc == RCH - 1),
                                    skip_group_check=True,
                                )
                        nc.scalar.copy(gsb[:], pgt[:])
                        for h in range(2):
                            pgn = pm.tile([128, C], F32, tag="pt", bufs=2)
                            nc.tensor.transpose(pgn[:], gsb[:, 128 * h:128 * (h + 1)], id16x4[0:C, :])
                            nc.scalar.copy(gacc[:, 2 * sb + h, :], pgn[:])
                        continue_sb0 = True
                    else:
                        continue_sb0 = False
                    for g in range(RCH // 8 if not continue_sb0 else 0):
                        u_t = up.tile([128, 8, 256], BF16, tag="u")
                        nc.sync.dma_start(
                            out=u_t[:],
                            in_=u_sh[1024 * g:1024 * (g + 1), 256 * sb:256 * (sb + 1)]
                            .rearrange("(j p) c -> p j c", p=128),
                        )
                        for jj in range(8):
                            rc = 8 * g + jj
                            q = rc % 2
                            if q == 0:
                                pt = pm.tile([128, 2, 2, 176], F32, tag="pt", bufs=2)
                            for h in range(2):
                                nc.tensor.matmul(
                                    pt[:, q, h, 0:C + 128],
                                    lhsT=u_t[:, jj, 128 * h:128 * (h + 1)],
                                    rhs=zid[:, rc, :],
                                    start=True, stop=True,
                                )
                            if q == 1:
                                dst = ut_t[:, rc - 1:rc + 1, :, :]
                                src = pt[:, :, :, 0:C + 128]
                                if (rc // 2) % 2 == 0:
                                    nc.scalar.copy(dst, src)
                                else:
                                    nc.vector.tensor_copy(dst, src)
                        gin = ut_t[:, 8 * g:8 * (g + 1), :, 0:C].transpose([0, 2, 3, 1])
                        if g == 0:
                            nc.vector.tensor_reduce(
                                out=gacc[:, 2 * sb:2 * sb + 2, :], in_=gin,
                                op=mybir.AluOpType.add, axis=mybir.AxisListType.X,
                            )
                        else:
                            gt = sp.tile([128, 2, C], F32, tag="gt")
                            nc.vector.tensor_reduce(
                                out=gt[:], in_=gin,
                                op=mybir.AluOpType.add, axis=mybir.AxisListType.X,
                            )
                            nc.vector.tensor_add(gacc[:, 2 * sb:2 * sb + 2, :],
                                                 gacc[:, 2 * sb:2 * sb + 2, :], gt[:])
                    for h in range(2):
                        b = 2 * sb + h
                        y_t = sp.tile([128, C], BF16, tag="y")
                        nc.vector.tensor_mul(y_t[:], gacc[:, b, :], hacc[:, b, :])
                        for rgp in range(16):
                            kq, off = rgp // 4, (rgp % 4) * 512
                            nc.tensor.matmul(
                                pacc[32 * kq:32 * kq + C, off:off + 512],
                                lhsT=y_t[:],
                                rhs=ut_t[:, rgp * 4:(rgp + 1) * 4, h, C:C + 128],
                                start=(b % 4 == 0), stop=(b % 4 == 3),
                                skip_group_check=True,
                                tile_position=(0, 32 * kq),
                            )
                        if b == 3:
                            emit_acc_tail(pm, accsb_a, o8_a, rs_in_a)
                            nc.gpsimd.collective_compute(
                                "ReduceScatter", mybir.AluOpType.add, replica_groups=rg,
                                ins=[rs_in_a.opt()], outs=[rs_out_a.opt()],
                            )
                        if b == 7:
                            emit_acc_tail(pm, accsb_b, o8_b, rs_in_b)
                            nc.gpsimd.collective_compute(
                "ReduceScatter", mybir.AluOpType.add, replica_groups=rg,
                ins=[rs_in_b.opt()], outs=[rs_out_b.opt()],
            )
            nc.sync.dma_start(out=smin[:], in_=rs_out_a[:].rearrange("(j p) c -> p j c", p=128))
            nc.sync.dma_start(out=smb[:], in_=rs_out_b[:].rearrange("(j p) c -> p j c", p=128))
            nc.vector.tensor_add(smf[:], smin[:], smb[:])
            mnegs = sp.tile([128, MYR], F32, tag="mneg")
            ssum = sp.tile([128, MYR], F32, tag="ssum")
            lns = sp.tile([128, MYR], F32, tag="lns")
            for rc in range(MYR):
                s = smf[:, rc, :]
                nc.vector.tensor_reduce(out=mnegs[:, rc:rc + 1], in_=s,
                                        op=mybir.AluOpType.max,
                                        axis=mybir.AxisListType.X, negate=True)
                et = sp.tile([128, C], F32, tag="et")
                nc.scalar.activation(et[:], s, mybir.ActivationFunctionType.Exp,
                                     bias=mnegs[:, rc:rc + 1], scale=1.0,
                                     accum_out=ssum[:, rc:rc + 1])
            nc.scalar.activation(lns[:], ssum[:], mybir.ActivationFunctionType.Ln)
            for rc in range(MYR):
                nc.vector.tensor_scalar(
                    out=smout[:, rc, :], in0=smf[:, rc, :],
                    scalar1=mnegs[:, rc:rc + 1], scalar2=lns[:, rc:rc + 1],
                    op0=mybir.AluOpType.add, op1=mybir.AluOpType.subtract,
                )
            nc.scalar.dma_start(
                out=out_sh[:].rearrange("(j p) c -> p j c", p=128), in_=smout[:])

    nc.compile()
    return nc


def _prep_inputs(origin_e, U, x, W1, b1, W2, b2, temp):
    origin_e = np.ascontiguousarray(np.asarray(origin_e, np.float32))
    U = np.asarray(U, np.float32)
    x = np.asarray(x, np.float32)
    W1 = np.asarray(W1, np.float32)
    b1 = np.asarray(b1, np.float32)
    W2 = np.asarray(W2, np.float32)
    b2 = np.asarray(b2, np.float32)

    jc = _jacobi_coef_rows(temp)
    id128 = np.eye(128, dtype=NPBF16)
    id16 = np.zeros((128, C), np.float32)
    for k in range(4):
        id16[32 * k:32 * k + C, :] = np.eye(C, dtype=np.float32)
    idb16 = id16.astype(NPBF16)
    w1r = np.ascontiguousarray(
        W1.reshape(4, 128, HID).transpose(1, 0, 2).reshape(128, 4 * HID)).astype(NPBF16)
    shared = {
        "w1r": w1r, "w2d": np.ascontiguousarray(W2).astype(NPBF16),
        "b1c": np.ascontiguousarray(b1.reshape(HID, 1)),
        "b2c": np.ascontiguousarray(b2.reshape(C, 1)),
        "jcd": jc, "id128d": id128, "id16x4d": id16, "idb16d": idb16,
        "x_shard": np.ascontiguousarray(x.T).astype(NPBF16),
    }
    in_maps = []
    for i in range(NCORES):
        m = dict(shared)
        m["u_shard"] = np.ascontiguousarray(U[:, i * SH:(i + 1) * SH]).astype(NPBF16)
        m["e_shard"] = np.ascontiguousarray(
            origin_e[i * SH:(i + 1) * SH].reshape(MYR, 128))
        in_maps.append(m)
    return in_maps


def _get_program():
    if "nc" not in _CACHE:
        _CACHE["nc"] = _build()
    return _CACHE["nc"]


def run(inputs, trace=False, **kw):
    nc = _get_program()
    in_maps = _prep_inputs(**inputs)
    res = run_bass_kernel_spmd(nc, in_maps, core_ids=list(range(NCORES)),
                               trace=trace, **kw)
    out = np.concatenate([res.results[i]["out_shard"] for i in range(NCORES)], axis=0)
    return out, res


def kernel(origin_e, U, x, W1, b1, W2, b2, temp):
    out, _ = run(dict(origin_e=origin_e, U=U, x=x, W1=W1, b1=b1, W2=W2,
                      b2=b2, temp=temp))
    return out


# revision 31
# speedup vs baseline: 1.2649x; 1.2649x over previous
"""Iter-1 + G1 (pt 4-slot 176-stride + incremental G reduce)."""

import os
import sys

import numpy as np

for _p in ("/opt/trn_rl_repo", "/root/.axon_site/_ro/trn_rl_repo"):
    if os.path.isdir(_p) and _p not in sys.path:
        sys.path.insert(0, _p)

import ml_dtypes

import concourse.bacc as bacc
import concourse.bass as bass  # noqa: F401
import concourse.mybir as mybir
import concourse.tile as tile
from concourse.bass_utils import run_bass_kernel_spmd

F32 = mybir.dt.float32
BF16 = mybir.dt.bfloat16
NPBF16 = ml_dtypes.bfloat16
N, F_IN, HID, C, K = 8192, 512, 64, 16, 10
BASE_ALPHA = 0.5
JA, JB, JL, JR = 1.0, 1.0, -1.0, 1.0
NCORES = 8
SH = N // NCORES
NB = SH // 128
NSB = NB // 2
RCH = N // 128
MYR = SH // 128

_CACHE = {}

def _jacobi_coef_rows(temp):
    """Host-precomputed per-channel coefficient rows, [30*C] packed."""
    a, b, l, r = JA, JB, JL, JR
    alphas = (BASE_ALPHA * np.tanh(np.asarray(temp, np.float64)))  # [C, K+1]
    rows = [alphas[:, 0]]
    coef1 = (a - b) / 2 - (a + b + 2) / 2 * (l + r) / (r - l)
    coef2 = (a + b + 2) / (r - l)
    rows.append(coef1 * alphas[:, 1])   # c1_0
    rows.append(coef2 * alphas[:, 1])   # c1_1
    for L in range(2, K + 1):
        coef_l = 2 * L * (L + a + b) * (2 * L - 2 + a + b)
        c_lm1_1 = (2 * L + a + b - 1) * (2 * L + a + b) * (2 * L + a + b - 2)
        c_lm1_2 = (2 * L + a + b - 1) * (a ** 2 - b ** 2)
        c_lm2 = 2 * (L - 1 + a) * (L - 1 + b) * (2 * L + a + b)
        tmp1 = alphas[:, L - 1] * (c_lm1_1 / coef_l)
        tmp2 = alphas[:, L - 1] * (c_lm1_2 / coef_l)
        tmp3 = alphas[:, L - 1] * alphas[:, L - 2] * (c_lm2 / coef_l)
        rows.append(tmp1 * (2 / (r - l)))                    # t1
        rows.append(tmp1 * ((r + l) / (r - l)) + tmp2)       # t2
        rows.append(tmp3)                                    # t3
    packed = np.concatenate(rows).astype(np.float32).reshape(1, 30 * C)
    return np.ascontiguousarray(np.repeat(packed, 128, axis=0))


def _bc(ap, shape, axis=1):
    """Broadcast an AP to a 3D [128, NB, C]-style shape with stride-0 dims."""
    while ap.ndim < len(shape):
        ap = ap.unsqueeze(axis)
    return ap.broadcast_to(shape)




def _build():
    nc = bacc.Bacc("TRN2", target_bir_lowering=False, debug=False)

    u_sh = nc.dram_tensor("u_shard", [N, SH], BF16, kind="ExternalInput")
    x_sh = nc.dram_tensor("x_shard", [F_IN, N], BF16, kind="ExternalInput")
    e_sh = nc.dram_tensor("e_shard", [MYR, 128], F32, kind="ExternalInput")
    w1r = nc.dram_tensor("w1r", [128, 4 * HID], BF16, kind="ExternalInput")
    w2d = nc.dram_tensor("w2d", [HID, C], BF16, kind="ExternalInput")
    b1c = nc.dram_tensor("b1c", [HID, 1], F32, kind="ExternalInput")
    b2c = nc.dram_tensor("b2c", [C, 1], F32, kind="ExternalInput")
    jcd = nc.dram_tensor("jcd", [128, 30 * C], F32, kind="ExternalInput")
    id128d = nc.dram_tensor("id128d", [128, 128], BF16, kind="ExternalInput")
    id16x4d = nc.dram_tensor("id16x4d", [128, C], F32, kind="ExternalInput")
    idb16d = nc.dram_tensor("idb16d", [128, C], BF16, kind="ExternalInput")
    warmd = nc.dram_tensor("warmd", [1, C], F32, kind="ExternalInput")
    out_sh = nc.dram_tensor("out_shard", [SH, C], F32, kind="ExternalOutput")

    rg = [list(range(NCORES))]

    with nc.allow_low_precision(reason="bf16 single-pass matmul path"), \
         tile.TileContext(nc) as tc:
        with (
            tc.tile_pool(name="dram", bufs=1, space="DRAM") as dram,
            tc.tile_pool(name="consts", bufs=1) as cp,
            tc.tile_pool(name="persist", bufs=1) as pp,
            tc.tile_pool(name="usb", bufs=12) as up,
            tc.tile_pool(name="utsb", bufs=2) as utp,
            tc.tile_pool(name="small", bufs=4) as sp,
        ):
            warm_in = dram.tile([1, C], F32)
            warm_out = dram.tile([NCORES, C], F32, addr_space="Shared")
            rs_in_a = dram.tile([N, C], mybir.dt.float16)
            rs_in_b = dram.tile([N, C], mybir.dt.float16)
            rs_out_a = dram.tile([SH, C], mybir.dt.float16)
            rs_out_b = dram.tile([SH, C], mybir.dt.float16)

            # warm up the CC stream / cross-core barrier while phase 0 runs
            nc.gpsimd.dma_start(out=warm_in[:], in_=warmd[:])
            nc.gpsimd.collective_compute(
                "AllGather", mybir.AluOpType.bypass, replica_groups=rg,
                ins=[warm_in.opt()], outs=[warm_out.opt()],
            )

            id128 = cp.tile_from(id128d[:])
            id16x4 = cp.tile_from(id16x4d[:])
            idb16 = cp.tile_from(idb16d[:])
            jc = cp.tile_from(jcd[:])
            w1 = cp.tile_from(w1r[:])
            w2 = cp.tile_from(w2d[:])
            b1 = cp.tile_from(b1c[:])
            b2 = cp.tile_from(b2c[:])
            e_row = cp.tile_from(e_sh[:])

            zid = pp.tile([128, RCH, C + 128], BF16)
            gsb = pp.tile([16, 256], F32)
            gacc = pp.tile([128, NB, C], F32)
            e_col = pp.tile([128, MYR], F32)
            hacc = pp.tile([128, NB, C], F32)
            xs_a = pp.tile([128, NB, C], F32)
            xs_b = pp.tile([128, NB, C], F32)
            htmp = pp.tile([128, NB, C], F32)
            htmp2 = pp.tile([128, NB, C], F32)
            accsb_a = pp.tile([128, 2048], BF16)
            accsb_b = pp.tile([128, 2048], BF16)
            smin = pp.tile([128, MYR, C], mybir.dt.float16)
            smb = pp.tile([128, MYR, C], mybir.dt.float16)
            smf = pp.tile([128, MYR, C], F32)
            smout = pp.tile([128, MYR, C], F32)
            nc.gpsimd.dma_start(
                out=zid[:, :, C:C + 128],
                in_=id128[:].unsqueeze(1).broadcast_to((128, RCH, 128)))

            # sb0's U fetches first: no z dependency, keeps DMA + PE busy
            u0_t = []
            for g in range(RCH // 8):
                u_t = up.tile([128, 8, 256], BF16, tag="u")
                nc.sync.dma_start(
                    out=u_t[:],
                    in_=u_sh[1024 * g:1024 * (g + 1), 0:256]
                    .rearrange("(j p) c -> p j c", p=128),
                )
                u0_t.append(u_t)

            # replicated MLP: every core computes the FULL z locally and
            # writes it straight into zid -- no AllGather, no bounce DMAs
            with tc.tile_pool(name="ppre", bufs=1, space="PSUM") as ppre, \
                 tc.tile_pool(name="mlp", bufs=2) as mp:
                pet = ppre.tile([128, MYR], F32, tag="ptmp", bufs=3)
                nc.tensor.transpose(pet[:], e_row[:], id16x4[0:MYR, 0:MYR])
                nc.scalar.copy(e_col[:], pet[:])
                for ch in range(8):
                    xTq = mp.tile([128, 4, 1024], BF16, tag="xq")
                    nc.scalar.dma_start(
                        out=xTq[:],
                        in_=x_sh[:, 1024 * ch:1024 * (ch + 1)]
                        .rearrange("(a p) r -> p a r", p=128))
                    ph = ppre.tile([HID, 1024], F32, tag="ph", bufs=1)
                    for half in range(2):
                        for fb in range(4):
                            nc.tensor.matmul(
                                ph[:, 512 * half:512 * (half + 1)],
                                lhsT=w1[:, fb * HID:(fb + 1) * HID],
                                rhs=xTq[:, fb, 512 * half:512 * (half + 1)],
                                start=(fb == 0), stop=(fb == 3),
                            )
                    hq = mp.tile([HID, 1024], BF16, tag="hq")
                    nc.scalar.activation(hq[:], ph[:],
                                         mybir.ActivationFunctionType.Relu,
                                         bias=b1[:, 0:1], scale=1.0)
                    pz = ppre.tile([C, 1024], F32, tag="pz", bufs=1)
                    for half in range(2):
                        nc.tensor.matmul(pz[:, 512 * half:512 * (half + 1)],
                                         lhsT=w2[:],
                                         rhs=hq[:, 512 * half:512 * (half + 1)],
                                         start=True, stop=True)
                    zTq = mp.tile([C, 1024], F32, tag="zq")
                    nc.vector.tensor_scalar_add(zTq[:], pz[:], b2[:, 0:1])
                    for j in range(8):
                        rc = 8 * ch + j
                        pzt = ppre.tile([128, C], F32, tag="ptmp", bufs=3)
                        nc.tensor.transpose(pzt[:], zTq[:, 128 * j:128 * (j + 1)],
                                            id16x4[0:C, :])
                        if rc % 2 == 0:
                            nc.scalar.copy(zid[:, rc, 0:C], pzt[:])
                        else:
                            nc.vector.tensor_copy(zid[:, rc, 0:C], pzt[:])
            ev = _bc(e_col[:], (128, NB, C), axis=2)

            def jrow(i):
                return _bc(jc[:, i * C:(i + 1) * C], (128, NB, C))

            nc.gpsimd.tensor_copy(xs_a[:], jrow(0))
            nc.gpsimd.tensor_mul(htmp[:], xs_a[:], ev)
            nc.gpsimd.tensor_mul(htmp[:], htmp[:], jrow(2))
            nc.gpsimd.tensor_add(xs_b[:], htmp[:], jrow(1))
            nc.gpsimd.tensor_add(hacc[:], xs_a[:], xs_b[:])
            xm2, xm1 = xs_a, xs_b
            for L in range(2, K + 1):
                r0 = 3 + 3 * (L - 2)
                nc.gpsimd.tensor_mul(htmp[:], xm1[:], ev)
                nc.gpsimd.tensor_mul(htmp[:], htmp[:], jrow(r0))
                nc.gpsimd.tensor_mul(htmp2[:], xm1[:], jrow(r0 + 1))
                nc.gpsimd.tensor_sub(htmp[:], htmp[:], htmp2[:])
                nc.gpsimd.tensor_mul(htmp2[:], xm2[:], jrow(r0 + 2))
                nc.gpsimd.tensor_sub(xm2[:], htmp[:], htmp2[:])
                nc.gpsimd.tensor_add(hacc[:], hacc[:], xm2[:])
                xm2, xm1 = xm1, xm2

            def emit_acc_tail(pm, accsb, rs_in):
                for ch in range(4):
                    dstc = accsb[:, 512 * ch:512 * (ch + 1)]
                    if ch % 2 == 0:
                        nc.scalar.copy(dstc, pacc[:, 512 * ch:512 * (ch + 1)])
                    else:
                        nc.vector.tensor_copy(dstc, pacc[:, 512 * ch:512 * (ch + 1)])
                o8 = None
                for rc in range(RCH):
                    kq, jj = rc // 16, rc % 16
                    pt2 = pm.tile([128, 4, C + 128], BF16, tag="pt", bufs=2)
                    nc.tensor.transpose(
                        pt2[:, 0, 0:C],
                        accsb[32 * kq:32 * kq + C, jj * 128:(jj + 1) * 128],
                        idb16[32 * kq:32 * kq + C, :],
                        tile_position=(32 * kq, 0),
                    )
                    if rc % 8 == 0:
                        o8 = sp.tile([128, 8, C], mybir.dt.float16, tag="o8")
                    dst = o8[:, rc % 8, :]
                    if rc % 2 == 0:
                        nc.scalar.copy(dst, pt2[:, 0, 0:C])
                    else:
                        nc.vector.tensor_copy(dst, pt2[:, 0, 0:C])
                    if rc % 8 == 7:
                        r0 = rc - 7
                        nc.gpsimd.dma_start(
                            out=rs_in[r0 * 128:(r0 + 8) * 128, :]
                            .rearrange("(j p) c -> p j c", p=128),
                            in_=o8[:],
                        )

            with (
                tc.tile_pool(name="pmain", bufs=1, space="PSUM") as pm,
            ):
                pacc = pm.tile([128, 2048], F32, tag="pacc")
                nc.vector.memset(pacc[:], 0.0)
                for sb in range(NSB):
                    ut_t = utp.tile([128, RCH, 2, C + 128], BF16, tag="ut")
                    if sb == 0:
                        # pass T: transpose-only (no z dependency)
                        for g in range(RCH // 8):
                            u_t = u0_t[g]
                            for jj in range(8):
                                rc = 8 * g + jj
                                q = rc % 2
                                if q == 0:
                                    ptb = pm.tile([128, 2, 2, 176], BF16,
                                                  tag="pt", bufs=2)
                                for h in range(2):
                                    nc.tensor.transpose(
                                        ptb[:, q, h, 0:128],
                                        u_t[:, jj, 128 * h:128 * (h + 1)],
                                        id128[:],
                                    )
                                if q == 1:
                                    dstb = ut_t[:, rc - 1:rc + 1, :, C:C + 128]
                                    srcb = ptb[:, :, :, 0:128]
                                    if (rc // 2) % 2 == 0:
                                        nc.scalar.copy(dstb, srcb)
                                    else:
                                        nc.vector.tensor_copy(dstb, srcb)
                        # pass G (after z lands): z-stationary accumulation
                        # single 256-wide group: a start-MM's has_written clear
                        # wipes its partition rows across the whole bank, so
                        # both blocks must share one accumulation region
                        pgt = pm.tile([16, 256], F32, tag="pt", bufs=2)
                        for g in range(RCH // 8):
                            u_t = u0_t[g]
                            for jj in range(8):
                                rc = 8 * g + jj
                                nc.tensor.matmul(
                                    pgt[:],
                                    lhsT=zid[:, rc, 0:C],
                                    rhs=u_t[:, jj, :],
                                    start=(rc == 0), stop=(rc == RCH - 1),
                                    skip_group_check=True,
                                )
                        nc.scalar.copy(gsb[:], pgt[:])
                        for h in range(2):
                            pgn = pm.tile([128, C], F32, tag="pt", bufs=2)
                            nc.tensor.transpose(pgn[:], gsb[:, 128 * h:128 * (h + 1)], id16x4[0:C, :])
                            nc.scalar.copy(gacc[:, 2 * sb + h, :], pgn[:])
                        continue_sb0 = True
                    else:
                        continue_sb0 = False
                    for g in range(RCH // 8 if not continue_sb0 else 0):
                        u_t = up.tile([128, 8, 256], BF16, tag="u")
                        nc.sync.dma_start(
                            out=u_t[:],
                            in_=u_sh[1024 * g:1024 * (g + 1), 256 * sb:256 * (sb + 1)]
                            .rearrange("(j p) c -> p j c", p=128),
                        )
                        for jj in range(8):
                            rc = 8 * g + jj
                            q = rc % 2
                            if q == 0:
                                pt = pm.tile([128, 2, 2, 176], F32, tag="pt", bufs=2)
                            for h in range(2):
                                nc.tensor.matmul(
                                    pt[:, q, h, 0:C + 128],
                                    lhsT=u_t[:, jj, 128 * h:128 * (h + 1)],
                                    rhs=zid[:, rc, :],
                                    start=True, stop=True,
                                )
                            if q == 1:
                                dst = ut_t[:, rc - 1:rc + 1, :, :]
                                src = pt[:, :, :, 0:C + 128]
                                if (rc // 2) % 2 == 0:
                                    nc.scalar.copy(dst, src)
                                else:
                                    nc.vector.tensor_copy(dst, src)
                        gin = ut_t[:, 8 * g:8 * (g + 1), :, 0:C].transpose([0, 2, 3, 1])
                        if g == 0:
                            nc.vector.tensor_reduce(
                                out=gacc[:, 2 * sb:2 * sb + 2, :], in_=gin,
                                op=mybir.AluOpType.add, axis=mybir.AxisListType.X,
                            )
                        else:
                            gt = sp.tile([128, 2, C], F32, tag="gt")
                            nc.vector.tensor_reduce(
                                out=gt[:], in_=gin,
                                op=mybir.AluOpType.add, axis=mybir.AxisListType.X,
                            )
                            nc.vector.tensor_add(gacc[:, 2 * sb:2 * sb + 2, :],
                                                 gacc[:, 2 * sb:2 * sb + 2, :], gt[:])
                    for h in range(2):
                        b = 2 * sb + h
                        y_t = sp.tile([128, C], BF16, tag="y")
                        nc.vector.tensor_mul(y_t[:], gacc[:, b, :], hacc[:, b, :])
                        for rgp in range(16):
                            kq, off = rgp // 4, (rgp % 4) * 512
                            nc.tensor.matmul(
                                pacc[32 * kq:32 * kq + C, off:off + 512],
                                lhsT=y_t[:],
                                rhs=ut_t[:, rgp * 4:(rgp + 1) * 4, h, C:C + 128],
                                start=(b % 4 == 0), stop=(b % 4 == 3),
                                skip_group_check=True,
                                tile_position=(0, 32 * kq),
                            )
                        if b == 3:
                            emit_acc_tail(pm, accsb_a, rs_in_a)
                            nc.gpsimd.collective_compute(
                                "ReduceScatter", mybir.AluOpType.add, replica_groups=rg,
                                ins=[rs_in_a.opt()], outs=[rs_out_a.opt()],
                            )
                        if b == 7:
                            emit_acc_tail(pm, accsb_b, rs_in_b)
                            nc.gpsimd.collective_compute(
                "ReduceScatter", mybir.AluOpType.add, replica_groups=rg,
                ins=[rs_in_b.opt()], outs=[rs_out_b.opt()],
            )
            nc.sync.dma_start(out=smin[:], in_=rs_out_a[:].rearrange("(j p) c -> p j c", p=128))
            nc.sync.dma_start(out=smb[:], in_=rs_out_b[:].rearrange("(j p) c -> p j c", p=128))
            nc.vector.tensor_add(smf[:], smin[:], smb[:])
            mnegs = sp.tile([128, MYR], F32, tag="mneg")
            ssum = sp.tile([128, MYR], F32, tag="ssum")
            lns = sp.tile([128, MYR], F32, tag="lns")
            for rc in range(MYR):
                s = smf[:, rc, :]
                nc.vector.tensor_reduce(out=mnegs[:, rc:rc + 1], in_=s,
                                        op=mybir.AluOpType.max,
                                        axis=mybir.AxisListType.X, negate=True)
                et = sp.tile([128, C], F32, tag="et")
                nc.scalar.activation(et[:], s, mybir.ActivationFunctionType.Exp,
                                     bias=mnegs[:, rc:rc + 1], scale=1.0,
                                     accum_out=ssum[:, rc:rc + 1])
            nc.scalar.activation(lns[:], ssum[:], mybir.ActivationFunctionType.Ln)
            for rc in range(MYR):
                nc.vector.tensor_scalar(
                    out=smout[:, rc, :], in0=smf[:, rc, :],
                    scalar1=mnegs[:, rc:rc + 1], scalar2=lns[:, rc:rc + 1],
                    op0=mybir.AluOpType.add, op1=mybir.AluOpType.subtract,
                )
            nc.scalar.dma_start(
                out=out_sh[:].rearrange("(j p) c -> p j c", p=128), in_=smout[:])

    nc.compile()
    return nc


def _prep_inputs(origin_e, U, x, W1, b1, W2, b2, temp):
    origin_e = np.ascontiguousarray(np.asarray(origin_e, np.float32))
    U = np.asarray(U, np.float32)
    x = np.asarray(x, np.float32)
    W1 = np.asarray(W1, np.float32)
    b1 = np.asarray(b1, np.float32)
    W2 = np.asarray(W2, np.float32)
    b2 = np.asarray(b2, np.float32)

    jc = _jacobi_coef_rows(temp)
    id128 = np.eye(128, dtype=NPBF16)
    id16 = np.zeros((128, C), np.float32)
    for k in range(4):
        id16[32 * k:32 * k + C, :] = np.eye(C, dtype=np.float32)
    idb16 = id16.astype(NPBF16)
    w1r = np.ascontiguousarray(
        W1.reshape(4, 128, HID).transpose(1, 0, 2).reshape(128, 4 * HID)).astype(NPBF16)
    shared = {
        "w1r": w1r, "w2d": np.ascontiguousarray(W2).astype(NPBF16),
        "b1c": np.ascontiguousarray(b1.reshape(HID, 1)),
        "b2c": np.ascontiguousarray(b2.reshape(C, 1)),
        "jcd": jc, "id128d": id128, "id16x4d": id16, "idb16d": idb16,
        "x_shard": np.ascontiguousarray(x.T).astype(NPBF16),
        "warmd": np.zeros((1, C), np.float32),
    }
    in_maps = []
    for i in range(NCORES):
        m = dict(shared)
        m["u_shard"] = np.ascontiguousarray(U[:, i * SH:(i + 1) * SH]).astype(NPBF16)
        m["e_shard"] = np.ascontiguousarray(
            origin_e[i * SH:(i + 1) * SH].reshape(MYR, 128))
        in_maps.append(m)
    return in_maps


def _get_program():
    if "nc" not in _CACHE:
        _CACHE["nc"] = _build()
    return _CACHE["nc"]


def run(inputs, trace=False, **kw):
    nc = _get_program()
    in_maps = _prep_inputs(**inputs)
    res = run_bass_kernel_spmd(nc, in_maps, core_ids=list(range(NCORES)),
                               trace=trace, **kw)
    out = np.concatenate([res.results[i]["out_shard"] for i in range(NCORES)], axis=0)
    return out, res


def kernel(origin_e, U, x, W1, b1, W2, b2, temp):
    out, _ = run(dict(origin_e=origin_e, U=U, x=x, W1=W1, b1=b1, W2=W2,
                      b2=b2, temp=temp))
    return out
